# revision 28
# baseline (speedup 1.0000x reference)
"""AUCM loss kernel for Trainium2 (8 NeuronCores, raw Bass).

Reference math (N = 16384 preds, int32 targets):
    pos = preds[targets==1]; neg = preds[targets==0]
    d_ij = 1 - (pos_i - neg_j)
    loss = mean_ij [ d_ij^2 + MARGIN*relu(d_ij) ]

Decomposition: with u_i = 1 - pos_i and v_j = neg_j, d_ij = u_i + v_j.
    sum_ij d^2     = Nv*sum(u^2) + 2*sum(u)*sum(v) + Nu*sum(v^2)  (host, O(N))
    sum_ij relu(d) = the real O(Nu*Nv) work -> computed on device.

Device strategy (no TensorEngine; ScalarE and VectorE both stream the
pairwise grid directly out of SBUF — an explicit PE-built D matrix would cap
both consumers at the PE's own column rate):
  - v is DMA-broadcast to all 128 partitions: v_rep [128, q] fp32 (striped
    over 8 DMA queues).
  - A 128-row block of u lives as one column u_col [128,1].
  - ScalarE, one instruction per (block, chunk):
        ACTIVATE(Relu, in=v_rep, bias=u_col, accum_out) ->
        per-partition sum_j relu(v_j + u_p); -1e30 padding (rows or cols)
        makes relu() return 0 for any padded pair.
  - VectorE, one instruction per (block, chunk), using
        relu(v + u) = u + max(v, -u):
        TENSOR_SCALAR(max, scalar1=-u_col, reduce=add, accum_out) ->
        per-partition sum_j max(v_j, -u_p). The host adds the fd*u_p
        correction in float64 (padded cols contribute max(-1e30,-u) = -u,
        cancelled exactly by +u; padded rows are dropped on the host).
  - Each unit's accum lands in its own column of acc_a/acc_d; both matrices
    are DMA'd out raw and the host does the final combine.

The kernel is raw Bass (no TileContext): a 3-engine pipeline with two
semaphores (dma_in, acc_done). This avoids Tile's multi-microsecond
semaphore-init preamble and end-of-kernel barrier butterfly.

Sharding: the longer of (pos, neg) becomes the row side, split evenly across
the 8 cores (each core gets nblk 128-row blocks); the col side is replicated.
"""

import math
import os
import sys

import numpy as np

for _p in ("/opt/trn_rl_repo", "/root/.axon_site/_ro/trn_rl_repo"):
    if os.path.isdir(_p) and _p not in sys.path:
        sys.path.append(_p)

import concourse.bacc as bacc
import concourse.bass as bass
from concourse import mybir
from concourse.bass_utils import run_bass_kernel_spmd

N_CORES = 8
MARGIN = 1.0
NEG_BIG = -1.0e30
CHUNK = 4096  # max free-dim per consumer instruction

# test-harness hooks (the grading path never touches these)
TRACE = False
LAST_EXEC_NS = None
LAST_RESULTS = None

_prog_cache: dict = {}


def _chunks(q):
    out = []
    c0 = 0
    while c0 < q:
        fd = min(CHUNK, q - c0)
        out.append((c0, fd))
        c0 += fd
    return out


def _units(nblk, q):
    """(block, c0, fd) units; guarantees at least one unit per engine."""
    chunks = _chunks(q)
    units = [(b, c0, fd) for b in range(nblk) for c0, fd in chunks]
    if len(units) == 1:
        b, c0, fd = units[0]
        h = max(2, fd // 2) // 2 * 2  # even split
        units = [(b, c0, h), (b, c0 + h, fd - h)]
    return units


def _act_cost(fd):
    return (250.0 + fd) / 1.2 + 181.0  # ACTIVATE + READ_ACCUMULATOR (measured)


def _dve_cost(fd):
    return (130.0 + fd) / 0.96  # measured


def _assign(raw_units):
    """Greedy ACT/DVE split by measured cost, then split one boundary unit
    to equalize the two engines' finish times.

    Returns list of (b, c0, fd, eng, idx) work items plus (na, nd).
    """
    load_a = load_d = 0.0
    tagged = []
    for b, c0, fd in raw_units:
        if load_a + _act_cost(fd) <= load_d + _dve_cost(fd):
            tagged.append([b, c0, fd, "A"])
            load_a += _act_cost(fd)
        else:
            tagged.append([b, c0, fd, "D"])
            load_d += _dve_cost(fd)
    # balance: split the heavier engine's last unit across both engines
    if abs(load_a - load_d) > 400.0:
        heavy = "A" if load_a > load_d else "D"
        k = max(i for i, t in enumerate(tagged) if t[3] == heavy)
        b, c0, fd, _ = tagged[k]
        if fd >= 256:
            base_a = load_a - (_act_cost(fd) if heavy == "A" else 0.0)
            base_d = load_d - (_dve_cost(fd) if heavy == "D" else 0.0)
            best = None
            for x in range(128, fd - 127, 64):  # x cols stay on ACT
                fa = base_a + _act_cost(x) + 200.0  # bias: ACT finishes first
                fdv = base_d + _dve_cost(fd - x)
                m = max(fa, fdv)
                if best is None or m < best[0]:
                    best = (m, x)
            if best is not None and best[0] < max(load_a, load_d) - 200.0:
                x = best[1]
                tagged[k : k + 1] = [
                    [b, c0, x, "A"],
                    [b, c0 + x, fd - x, "D"],
                ]
    na = nd = 0
    out = []
    for b, c0, fd, eng in tagged:
        if eng == "A":
            out.append((b, c0, fd, "A", na))
            na += 1
        else:
            out.append((b, c0, fd, "D", nd))
            nd += 1
    return out, na, nd


def _build(nblk, q):
    """Raw Bass program for one core: nblk 128-row blocks x q cols."""
    key = (nblk, q)
    if key in _prog_cache:
        return _prog_cache[key]

    f32 = mybir.dt.float32
    bf16 = mybir.dt.bfloat16
    work, na, nd = _assign(_units(nblk, q))
    assert na >= 1 and nd >= 1

    nc = bacc.Bacc(None, target_bir_lowering=False)
    cols_big = nc.dram_tensor("cols_big", [q], f32, kind="ExternalInput")
    uu_t = nc.dram_tensor("uu", [128, 2 * nblk], f32, kind="ExternalInput")
    out_t = nc.dram_tensor("out", [128, na + nd], f32, kind="ExternalOutput")

    # input DMA plan: v_rep stripes (issued from 4 different sequencers so the
    # ~600ns-per-DMA descriptor writes don't serialize) + one u-tile DMA
    nstripes = max(1, min(2, q // 128))
    sw = (q // nstripes + 127) // 128 * 128
    stripes = []
    c0 = 0
    while c0 < q:
        stripes.append((c0, min(sw, q - c0)))
        c0 += sw
    n_in = len(stripes) + 1

    # Each unit gets a private scratch slice (the engines' main outputs are
    # dead stores — only accum_out matters — but same-engine WAW reuse is
    # unsafe on deep pipelines and trips the race detector).
    offs = []
    scr_w = 1  # slot 0 reserved for the table-load dummy
    for _b, _c0, fd, _eng, _idx in work:
        offs.append(scr_w)
        scr_w += fd

    with (
        nc.sbuf_tensor([128, q], f32) as v_rep,
        nc.sbuf_tensor([128, 2 * nblk], f32) as uu_sb,
        nc.sbuf_tensor([128, na + nd], f32) as acc,
        nc.sbuf_tensor([128, scr_w], bf16) as scr,
        nc.semaphore("dma_in") as dma_in,
        nc.semaphore("act_done") as act_done,
        nc.semaphore("dve_done") as dve_done,
        nc.Block() as block,
    ):
        ub_sb = uu_sb[:, :nblk]
        negu_sb = uu_sb[:, nblk:]
        h = cols_big[:]

        def stripe_dma(eng, s, sem):
            c0, w = stripes[s]
            bc = bass.AP(tensor=h.tensor, offset=h.offset + c0, ap=[[0, 128], [1, w]])
            eng.dma_start(out=v_rep[:, c0 : c0 + w], in_=bc).then_inc(sem, 16)

        # stripe issuers: only sync/scalar (HWDGE) and gpsimd (SWDGE) can DMA
        issuers = {}
        for s in range(len(stripes)):
            issuers.setdefault(["scalar", "sync"][s % 2], []).append(s)

        def wait_inputs(eng):
            eng.wait_ge(dma_in, 16 * n_in)

        @block.sync
        def _(sync: bass.BassEngine):
            for s in issuers.get("sync", []):
                stripe_dma(sync, s, dma_in)
            with nc.allow_non_contiguous_dma(reason="tiny [128, 2*nblk] u tile"):
                sync.dma_start(out=uu_sb[:, :], in_=uu_t[:, :]).then_inc(dma_in, 16)
            # stream results out as each consumer finishes (ACT is biased to
            # finish first)
            sync.wait_ge(act_done, 1)
            with nc.allow_non_contiguous_dma(reason="small accum outputs"):
                sync.dma_start(out=out_t[:, :na], in_=acc[:, :na]).then_inc(dma_in, 16)
            sync.wait_ge(dve_done, 1)
            with nc.allow_non_contiguous_dma(reason="small accum outputs"):
                sync.dma_start(out=out_t[:, na:], in_=acc[:, na:]).then_inc(dma_in, 16)
            sync.wait_ge(dma_in, 16 * (n_in + 2))

        @block.scalar
        def _(scalar: bass.BassEngine):
            # dummy activation: hoists the ~1.5us ACT_TABLE_LOAD before the
            # DMA wait so it overlaps the input transfer
            zero = nc.const_aps.scalar_like(0.0, scr[:, 0:1])
            scalar.activation(scr[:, 0:1], zero, mybir.ActivationFunctionType.Relu)
            for s in issuers.get("scalar", []):
                stripe_dma(scalar, s, dma_in)
            wait_inputs(scalar)
            seen = 0
            for k, (b, c0, fd, eng, idx) in enumerate(work):
                if eng != "A":
                    continue
                seen += 1
                ins = scalar.activation(
                    scr[:, offs[k] : offs[k] + fd],
                    v_rep[:, c0 : c0 + fd],
                    mybir.ActivationFunctionType.Relu,
                    bias=ub_sb[:, b : b + 1],
                    accum_out=acc[:, idx : idx + 1],
                )
                if seen == na:
                    ins.then_inc(act_done, 1)

        @block.vector
        def _(vector: bass.BassEngine):
            wait_inputs(vector)
            seen = 0
            for k, (b, c0, fd, eng, idx) in enumerate(work):
                if eng != "D":
                    continue
                seen += 1
                ins = vector.tensor_scalar(
                    scr[:, offs[k] : offs[k] + fd],
                    v_rep[:, c0 : c0 + fd],
                    negu_sb[:, b : b + 1],
                    None,
                    op0=mybir.AluOpType.max,
                    op1=mybir.AluOpType.add,
                    accum_out=acc[:, na + idx : na + idx + 1],
                )
                if seen == nd:
                    ins.then_inc(dve_done, 1)

    nc.finalize()
    _prog_cache[key] = (nc, work, na, nd)
    return _prog_cache[key]


def kernel(preds: np.ndarray, targets: np.ndarray) -> np.ndarray:
    global LAST_EXEC_NS, LAST_RESULTS

    p = np.asarray(preds, dtype=np.float32).reshape(-1)
    t = np.asarray(targets).reshape(-1)

    u = (1.0 - p[t == 1]).astype(np.float32)  # positive side
    v = p[t == 0].astype(np.float32)  # negative side
    nu, nv = u.size, v.size

    # Pick the row side (sharded across cores) to minimize per-core pair count.
    def cost(nrows, ncols):
        nblk = max(1, math.ceil(nrows / (128 * N_CORES)))
        q = max(128, 128 * math.ceil(ncols / 128))
        return nblk * 128 * q, nblk, q

    cost_u, nblk_u, q_u = cost(nu, nv)
    cost_v, nblk_v, q_v = cost(nv, nu)
    if cost_u <= cost_v:
        rows, cols, nblk, q = u, v, nblk_u, q_u
        n_rows_real, n_cols_real = nu, nv
    else:
        rows, cols, nblk, q = v, u, nblk_v, q_v
        n_rows_real, n_cols_real = nv, nu

    rtot = nblk * 128 * N_CORES
    nreal = rows.size
    rows_pad = np.zeros(rtot, dtype=np.float32)
    rows_pad[:nreal] = rows
    cols_big = np.full(q, NEG_BIG, dtype=np.float32)
    cols_big[: cols.size] = cols

    # per-core [128, nblk] layouts: element (p, b) = row b*128 + p of the slice
    ub_all = np.full(rtot, NEG_BIG, dtype=np.float32)
    ub_all[:nreal] = rows
    negu_all = np.zeros(rtot, dtype=np.float32)
    negu_all[:nreal] = -rows

    (nc, work, na, nd) = _build(nblk, q)

    per = nblk * 128
    in_maps = []
    for c in range(N_CORES):
        sl = slice(c * per, (c + 1) * per)
        uu = np.concatenate(
            [ub_all[sl].reshape(nblk, 128).T, negu_all[sl].reshape(nblk, 128).T],
            axis=1,
        )
        in_maps.append(
            {
                "cols_big": cols_big,
                "uu": np.ascontiguousarray(uu),
            }
        )

    br = run_bass_kernel_spmd(nc, in_maps, list(range(N_CORES)), trace=TRACE)
    results = br.results
    LAST_EXEC_NS = getattr(br, "exec_time_ns", None)
    LAST_RESULTS = br

    relu_sum = 0.0
    rows64 = rows_pad.astype(np.float64)
    for c in range(N_CORES):
        o = np.asarray(results[c]["out"], dtype=np.float64)
        acc_a, acc_d = o[:, :na], o[:, na:]
        base = c * per
        for b, c0, fd, eng, idx in work:
            lo = base + b * 128
            n_real_p = min(max(nreal - lo, 0), 128)
            if eng == "A":
                # padded rows/cols contribute exactly 0
                relu_sum += acc_a[:, idx].sum()
            elif n_real_p > 0:
                # sum over real rows of (acc + fd*u_p); padded cols inside
                # acc contribute -u_p each, cancelled exactly by +fd*u_p
                seg = acc_d[:n_real_p, idx]
                useg = rows64[lo : lo + n_real_p]
                relu_sum += seg.sum() + fd * useg.sum()

    u64 = u.astype(np.float64)
    v64 = v.astype(np.float64)
    sq_sum = (
        nv * (u64 * u64).sum() + 2.0 * u64.sum() * v64.sum() + nu * (v64 * v64).sum()
    )
    num_pairs = np.float64(nu) * np.float64(nv)
    with np.errstate(divide="ignore", invalid="ignore"):
        loss = np.float32((sq_sum + MARGIN * relu_sum) / num_pairs)
    return np.asarray(loss, dtype=np.float32)
